# revision 86
# baseline (speedup 1.0000x reference)
"""Trainium2 Bass kernel for nn_AutoRegressive (dense transformer decoder), v2.

Model: B=4 packed text+audio sequences, L=768, D=1024, 16 heads, DFF=4096,
6 norm-first decoder layers (self-attn w/ prefix-LM mask, cross-attn to the
packed embedding, FFN), weight-tied audio head. fp32 inputs/outputs.

Sharding: DP4 x TP2 over 8 cores. Core pair (2i, 2i+1) owns batch item i;
within a pair the 16 heads split 8+8 and DFF splits 2048+2048. Three pair-
AllReduces per layer; CA K/V projections are scheduled into AR shadows.

v2 design vs baseline:
- all weights pre-transposed + bf16 on host (no PE transposes, no ACT copies)
- activations bf16 (residual stream f32), matmuls bf16 @ 1 cyc/row
- embedding + positional encoding gathered/packed on host -> x0 DMA
- V projected token-major directly (stationary = activation tile), so AV
  stationary needs no on-device transpose; ones column gives softmax denom
- SA score tiles that are fully masked are skipped (12 of 18 (tk,chunk))
- mask applied multiplicatively AFTER exp (bf16 x bf16 -> 4x DVE rate)
- projections stream weights in L-chunks of 384 with k-outer/o-inner loops,
  FFN is fused (w1 block -> relu -> w2 block) to bound PSUM/SBUF residency
"""
import os
import numpy as np
import ml_dtypes

import concourse.bass as bass
from concourse import bacc
import concourse.mybir as mybir
import concourse.tile as tile
from concourse.bass_utils import run_bass_kernel_spmd

F32 = mybir.dt.float32
F32R = mybir.dt.float32r
BF16 = mybir.dt.bfloat16
AF = mybir.ActivationFunctionType
OP = mybir.AluOpType

B, Tt, Ta, L, D, H, DH, DFF, NL = 4, 128, 640, 768, 1024, 16, 64, 4096, 6
VT, VA = 256, 1026
NLAYERS = int(os.environ.get("KERNEL_NL", str(NL)))
P = 128
NT = L // P          # 6 sequence tiles
DK = D // P          # 8 feature tiles
QO = 4               # local q/k/ctx tiles (512 dims)
OO = 8               # output tiles of D
HLOC = 8             # local heads
HEADO = 5            # head out-tiles (640-row padded vocab slab)
CHN = (0, 384, 768)  # L chunks for streamed projections
# SA (tk allowed) per 256-wide q chunk c (q tiles 2c, 2c+1)
SA_SET = {0: (0, 1), 1: (0, 1, 2, 3), 2: (0, 1, 2, 3, 4, 5)}


def _build_nc():
    nc = bacc.Bacc(None)

    x0_d = nc.declare_dram_parameter("x0", [D, L], F32, isOutput=False)
    mem_d = nc.declare_dram_parameter("mem0", [D, L], BF16, isOutput=False)
    mask_d = nc.declare_dram_parameter("maskT", [P, NT * L], BF16, isOutput=False)
    wqk_sa = nc.declare_dram_parameter("wqk_sa", [NLAYERS, D, 1024], BF16, isOutput=False)
    wv_sa = nc.declare_dram_parameter("wv_sa", [NLAYERS, D, 512], BF16, isOutput=False)
    wo_sa = nc.declare_dram_parameter("wo_sa", [NLAYERS, 512, D], BF16, isOutput=False)
    wq_ca = nc.declare_dram_parameter("wq_ca", [NLAYERS, D, 512], BF16, isOutput=False)
    wk_ca = nc.declare_dram_parameter("wk_ca", [NLAYERS, D, 512], BF16, isOutput=False)
    wv_ca = nc.declare_dram_parameter("wv_ca", [NLAYERS, D, 512], BF16, isOutput=False)
    wo_ca = nc.declare_dram_parameter("wo_ca", [NLAYERS, 512, D], BF16, isOutput=False)
    w1_d = nc.declare_dram_parameter("w1T", [NLAYERS, D, 2048], BF16, isOutput=False)
    w2_d = nc.declare_dram_parameter("w2T", [NLAYERS, 2048, D], BF16, isOutput=False)
    head_d = nc.declare_dram_parameter("headT", [D, HEADO * P], BF16, isOutput=False)
    logits = nc.declare_dram_parameter("logits", [HEADO * P, L], BF16, isOutput=True)

    DBG = bool(int(os.environ.get("KERNEL_DEBUG", "0")))
    dbg = {}
    if DBG:
        for nm, shp, dt in [("dh1", [D, L], BF16), ("dqk", [1024, L], BF16),
                            ("dctx", [512, L], BF16), ("dx1", [D, L], F32),
                            ("dcak", [512, L], BF16), ("dctx2", [512, L], BF16),
                            ("dx2", [D, L], F32), ("dx3", [D, L], F32)]:
            dbg[nm] = nc.declare_dram_parameter(nm, shp, dt, isOutput=True)

    cc_in = nc.dram_tensor("cc_in", [D, L], BF16)
    cc_out = nc.dram_tensor("cc_out", [D, L], BF16)
    GROUPS = [[0, 1], [2, 3], [4, 5], [6, 7]]

    from contextlib import ExitStack
    with tile.TileContext(nc) as tc, ExitStack() as S:
        state = S.enter_context(tc.tile_pool(name="state", bufs=1))
        wpool = S.enter_context(tc.tile_pool(name="wpool", bufs=18))
        evb = S.enter_context(tc.tile_pool(name="evb", bufs=3))
        prb = S.enter_context(tc.tile_pool(name="prb", bufs=4))
        xsqp = S.enter_context(tc.tile_pool(name="xsqp", bufs=2))
        tmpp = S.enter_context(tc.tile_pool(name="tmpp", bufs=2))
        bcp = S.enter_context(tc.tile_pool(name="bcp", bufs=2))
        bcb = S.enter_context(tc.tile_pool(name="bcb", bufs=2))
        h1p = S.enter_context(tc.tile_pool(name="h1p", bufs=2))
        invp = S.enter_context(tc.tile_pool(name="invp", bufs=1))

        xf = state.tile([P, DK, L], F32)
        mem = state.tile([P, DK, L], BF16)
        hT = state.tile([P, DK, L], BF16)
        big = state.tile([P, OO, L], BF16)       # attn q(0:4) + k(4:8)
        h1b = state.tile([P, DK, L], BF16)       # FFN hidden tiles 8:16
        ctxT = state.tile([P, QO, L], BF16)
        vtok = state.tile([P, NT, HLOC, 66], BF16)
        mk = state.tile([P, NT, L], BF16)
        wvr_sa = state.tile([P, DK, 512], BF16)  # resident v weights
        wvr_ca = state.tile([P, DK, 512], BF16)
        onesb = state.tile([P, 1], BF16)
        epst = state.tile([1, 1], F32)
        mu_s = state.tile([1, L], F32)
        var_s = state.tile([1, L], F32)
        sd_s = state.tile([1, L], F32)
        sdb_s = state.tile([1, L], BF16)

        nc.vector.memset(onesb, 1.0)
        nc.vector.memset(epst, 1e-5)
        nc.vector.memset(vtok[:, :, :, 64:65], 1.0)
        for k in range(DK):
            nc.sync.dma_start(out=xf[:, k, :], in_=x0_d[k * P:(k + 1) * P, :])
            nc.sync.dma_start(out=mem[:, k, :], in_=mem_d[k * P:(k + 1) * P, :])
        for t in range(NT):
            nc.sync.dma_start(out=mk[:, t, :], in_=mask_d[:, t * L:(t + 1) * L])

        def dump(nm, tile_ap, n, off=0):
            if not DBG:
                return
            d = dbg[nm]
            for o in range(n):
                nc.sync.dma_start(out=d[o * P:(o + 1) * P, :],
                                  in_=tile_ap[:, off + o, :])

        # ---------------- helpers ----------------
        def layernorm():
            """LN over partition dim of xf -> hT (no affine; w=1, b=0)."""
            with tc.tile_pool(name="ln_ps", bufs=1, space="PSUM") as lps:
                s12 = lps.tile([33, L], F32)
                for k in range(DK):
                    xsq = xsqp.tile([P, 2, L], BF16, tag="xsq", name="xsq")
                    nc.scalar.copy(xsq[:, 0, :], xf[:, k, :])
                    # square the bf16 copy on the DVE (4x packed rate) instead
                    # of the scalar engine — ACT is the busier engine and also
                    # serves the attention exp
                    nc.vector.tensor_mul(out=xsq[:, 1, :], in0=xsq[:, 0, :],
                                         in1=xsq[:, 0, :])
                    st, sp = (k == 0), (k == DK - 1)
                    for c0, c1 in ((0, 512), (512, L)):
                        nc.tensor.matmul(s12[0:1, c0:c1], onesb,
                                         xsq[:, 0, c0:c1], start=st, stop=sp)
                        nc.tensor.matmul(s12[32:33, c0:c1], onesb,
                                         xsq[:, 1, c0:c1], start=st, stop=sp)
                nc.vector.tensor_scalar_mul(mu_s, s12[0:1, :], 1.0 / D)
                nc.vector.tensor_mul(out=var_s, in0=mu_s, in1=mu_s)
                nc.vector.scalar_tensor_tensor(
                    out=var_s, in0=s12[32:33, :], scalar=1.0 / D,
                    in1=var_s, op0=OP.mult, op1=OP.subtract)
            nc.scalar.activation(sd_s, var_s, AF.Sqrt, bias=epst[0:1, 0:1])
            nc.vector.reciprocal_approx_fast(out=var_s, in_=sd_s)
            nc.scalar.copy(sdb_s, var_s)
            mub = bcp.tile([P, L], F32, tag="mub", name="mub")
            nc.gpsimd.partition_broadcast(mub, mu_s[0:1, :])
            rb = bcb.tile([P, L], BF16, tag="rb", name="rb")
            nc.gpsimd.partition_broadcast(rb, sdb_s[0:1, :])
            for k in range(DK):
                t = tmpp.tile([P, L], BF16, tag="lt", name="lt")
                nc.vector.tensor_tensor(out=t, in0=xf[:, k, :], in1=mub,
                                        op=OP.subtract)
                nc.vector.tensor_mul(out=hT[:, k, :], in0=t, in1=rb)

        def proj(w_ap, n_o, n_k, wcols, rhs_fn, out_fn, o_map=None):
            """Streamed projection: out[o] = sum_k W^T[k]-tile @ rhs(k).

            k-outer / o-inner, full-L accumulators; the two L chunks share
            one stationary load. w_ap: [Din, wcols] dram slice, rhs_fn(k) ->
            bf16 AP [128, L]; out_fn(o, acc) with acc [128, L] f32 psum.
            o_map remaps weight-column position -> logical output index (the
            host packs q/k columns interleaved so attention can start after
            the first o-block).
            """
            with tc.tile_pool(name="pj_ps", bufs=1, space="PSUM") as pps:
                for ob0 in range(0, n_o, 4):
                    obn = min(4, n_o - ob0)
                    accs = [pps.tile([P, L], F32, tag=f"pacc{o}",
                                     name=f"pacc{o}") for o in range(obn)]
                    for k in range(n_k):
                        wt = wpool.tile([P, 1024], BF16, tag="w", name="wt")
                        nc.sync.dma_start(
                            out=wt[:, 0:obn * P],
                            in_=w_ap[k * P:(k + 1) * P,
                                     ob0 * P:(ob0 + obn) * P])
                        rhs = rhs_fn(k)
                        st, sp = (k == 0), (k == n_k - 1)
                        for o in range(obn):
                            w_sl = wt[:, o * P:(o + 1) * P]
                            nc.tensor.matmul(accs[o][:, 0:512], w_sl,
                                             rhs[:, 0:512], start=st, stop=sp)
                            nc.tensor.matmul(accs[o][:, 512:L], w_sl,
                                             rhs[:, 512:L], start=st, stop=sp)
                    for o in range(obn):
                        oi = ob0 + o
                        out_fn(o_map[oi] if o_map else oi, accs[o])

        def vproj(wvr, src, vdst, trange=range(NT)):
            """Token-major V: vdst[:, t, h, 0:64] = (src^T W_v^T)[t-tile]."""
            with tc.tile_pool(name="v_ps", bufs=2, space="PSUM") as vps:
                for t in trange:
                    vacc = vps.tile([P, DK, 64], F32, tag="vacc", name="vacc")
                    for k in range(DK):
                        nc.tensor.matmul(vacc[:, :, :],
                                         src[:, k, t * P:(t + 1) * P],
                                         wvr[:, k, :],
                                         start=(k == 0), stop=(k == DK - 1))
                    nc.vector.tensor_copy(out=vdst[:, t, :, 0:64],
                                          in_=vacc[:, :, :])

        def attention(masked, vtok, kt_fn):
            """softmax((big q)^T k / 8 + mask) @ v -> ctxT (denom via ones col).

            Per (head, key-tile): one stationary load for scores over the
            whole allowed q range, one exp, one mask multiply; AV shares the
            vtok stationary across both 512-boundary chunks. For SA, key
            tile tk only reaches queries q >= qlo(tk) (prefix text keys are
            all in tk 0), so higher tiles start at 256/512.
            """
            with tc.tile_pool(name="at_sps", bufs=2, space="PSUM") as sps, \
                 tc.tile_pool(name="at_cps", bufs=2, space="PSUM") as cps:
                for h in range(HLOC):
                    j, hb = h // 2, 64 * (h % 2)
                    ctx = cps.tile([P, L], F32, tag="ctx", name="ctx")
                    qlos = [((0, 0, 256, 256, 512, 512)[tk] if masked else 0)
                            for tk in range(NT)]
                    prs = {}

                    def score(tk):
                        qlo = qlos[tk]
                        sc = sps.tile([P, L], F32, tag="sc", name="sc")
                        kT = kt_fn(j)[hb:hb + 64, tk * P:(tk + 1) * P]
                        for c0, c1 in (((qlo, 512), (512, L)) if qlo < 512
                                       else ((512, L),)):
                            nc.tensor.matmul(sc[:, c0:c1], kT,
                                             big[hb:hb + 64, j, c0:c1],
                                             start=True, stop=True)
                        pr = prb.tile([P, L], BF16, tag="pr", name="pr")
                        nc.scalar.activation(pr[:, qlo:L], sc[:, qlo:L],
                                             AF.Exp, scale=0.125)
                        if masked:
                            nc.vector.tensor_mul(out=pr[:, qlo:L],
                                                 in0=pr[:, qlo:L],
                                                 in1=mk[:, tk, qlo:L])
                        prs[tk] = pr

                    def av(tk):
                        qlo = qlos[tk]
                        pr = prs.pop(tk)
                        for c0, c1 in (((qlo, 512), (512, L)) if qlo < 512
                                       else ((512, L),)):
                            nc.tensor.matmul(ctx[0:65, c0:c1],
                                             vtok[:, tk, h, 0:65],
                                             pr[:, c0:c1],
                                             start=(tk == 0),
                                             stop=(tk == NT - 1))

                    # software pipeline: keep one score in flight ahead of
                    # the AV so the PE never waits on the exp
                    score(0)
                    for tk in range(NT):
                        if tk + 1 < NT:
                            score(tk + 1)
                        av(tk)
                    den = invp.tile([1, L], F32, tag="den", name="den")
                    nc.scalar.copy(den, ctx[64:65, :])
                    inv1 = invp.tile([1, L], F32, tag="inv", name="inv1")
                    nc.vector.reciprocal_approx_fast(out=inv1, in_=den)
                    invb = bcp.tile([P, L], F32, tag="invb", name="invb")
                    nc.gpsimd.partition_broadcast(invb, inv1[0:1, :])
                    nc.vector.tensor_mul(out=ctxT[hb:hb + 64, j, :],
                                         in0=ctx[0:64, :], in1=invb[0:64, :])

        def out_evac(o, acc):
            ev = evb.tile([P, L], BF16, tag="ev", name="ev")
            nc.scalar.copy(ev, acc)
            nc.sync.dma_start(out=cc_in[o * P:(o + 1) * P, :], in_=ev)

        def out_evac_c(o, acc, c0, c1):
            ev = evb.tile([P, 384], BF16, tag="evc", name="evc")
            nc.scalar.copy(ev[:, 0:c1 - c0], acc[:, 0:c1 - c0])
            nc.sync.dma_start(out=cc_in[o * P:(o + 1) * P, c0:c1],
                              in_=ev[:, 0:c1 - c0])

        def ar_issue():
            nc.gpsimd.collective_compute(
                "AllReduce", OP.add, replica_groups=GROUPS,
                ins=[cc_in[:, :]], outs=[cc_out[:, :]])

        def ar_accum():
            for o in range(DK):
                rr = evb.tile([P, L], BF16, tag="rr", name="rr")
                nc.sync.dma_start(out=rr, in_=cc_out[o * P:(o + 1) * P, :])
                nc.vector.tensor_tensor(out=xf[:, o, :], in0=xf[:, o, :],
                                        in1=rr, op=OP.add)

        def qk_evac(o, acc):
            nc.vector.tensor_copy(out=big[:, o, :], in_=acc)

        def cak_evac(o, acc):
            nc.scalar.copy(big[:, 4 + o, :], acc)

        def relu_evac(o, acc):
            dst = big if o < DK else h1b
            nc.scalar.activation(dst[:, o % DK, :], acc, AF.Relu)

        def ffn():
            """w1 -> relu -> w2 as two paired-stationary projections; the
            2048 hidden dims stage into big (dead during FFN) + h1b."""
            proj(w1_d[l], 2 * DK, DK, 2048, lambda k: hT[:, k, :], relu_evac)
            proj(w2_d[l], OO, 2 * DK, 1024,
                 lambda k: (big if k < DK else h1b)[:, k % DK, :], out_evac)

        # ---------------- resident V weights for layer 0 ----------------
        def load_wv(wvr, w_ap):
            for k in range(DK):
                nc.sync.dma_start(out=wvr[:, k, :], in_=w_ap[k * P:(k + 1) * P, :])

        # ---------------- layers ----------------
        for l in range(NLAYERS):
            load_wv(wvr_sa, wv_sa[l])
            # ---- self-attention ----
            layernorm()
            if l == 0:
                dump("dh1", hT, DK)
            vproj(wvr_sa, hT, vtok)
            proj(wqk_sa[l], OO, DK, 1024, lambda k: hT[:, k, :], qk_evac,
                 o_map=[0, 4, 1, 5, 2, 6, 3, 7])
            if l == 0:
                dump("dqk", big, OO)
            attention(True, vtok, lambda j: big[:, 4 + j, :])
            if l == 0:
                dump("dctx", ctxT, QO)
            proj(wo_sa[l], OO, QO, 1024, lambda k: ctxT[:, k, :], out_evac)
            ar_issue()
            # AR1 shadow: CA k/v from mem (x-independent); k reuses big[4:8],
            # v reuses vtok (SA attention is done with both by this point)
            load_wv(wvr_ca, wv_ca[l])
            proj(wk_ca[l], QO, DK, 512, lambda k: mem[:, k, :], cak_evac)
            vproj(wvr_ca, mem, vtok)
            ar_accum()
            if l == 0:
                dump("dx1", xf, DK)

            # ---- cross-attention ----
            layernorm()
            proj(wq_ca[l], QO, DK, 512, lambda k: hT[:, k, :], qk_evac)
            if l == 0:
                dump("dcak", big, QO, off=4)
            attention(False, vtok, lambda j: big[:, 4 + j, :])
            if l == 0:
                dump("dctx2", ctxT, QO)
            proj(wo_ca[l], OO, QO, 1024, lambda k: ctxT[:, k, :], out_evac)
            ar_issue()
            ar_accum()
            if l == 0:
                dump("dx2", xf, DK)

            # ---- FFN ----
            layernorm()
            ffn()
            ar_issue()
            ar_accum()
            if l == 0:
                dump("dx3", xf, DK)

        # ---------------- head ----------------
        for k in range(DK):
            nc.scalar.copy(hT[:, k, :], xf[:, k, :])

        def head_evac(o, acc):
            ev = evb.tile([P, L], BF16, tag="ev", name="ev")
            nc.scalar.copy(ev, acc)
            nc.sync.dma_start(out=logits[o * P:(o + 1) * P, :], in_=ev)

        proj(head_d, HEADO, DK, HEADO * P, lambda k: hT[:, k, :], head_evac)

    nc.finalize()
    return nc


# ---------------------------------------------------------------------------
# host side
# ---------------------------------------------------------------------------

def _pe_table(length, d):
    pos = np.arange(length, dtype=np.float32)[:, None]
    div = np.exp(np.arange(0, d, 2, dtype=np.float32) * (-np.log(10000.0) / d))
    ang = pos * div
    out = np.zeros((length, d), np.float32)
    out[:, 0::2] = np.sin(ang)
    out[:, 1::2] = np.cos(ang)
    return out


BF = ml_dtypes.bfloat16


def _bf(a):
    return np.ascontiguousarray(np.asarray(a, np.float32).astype(BF))


_NC_CACHE = {}
LAST_RESULT = {}


def kernel(**inputs):
    f32 = lambda a: np.asarray(a, dtype=np.float32)
    text = np.asarray(inputs["text"]).astype(np.int64)
    audio = np.asarray(inputs["audio"]).astype(np.int64)
    tl = np.asarray(inputs["text_len_batch"]).astype(np.int64)
    al = np.asarray(inputs["audio_len_batch"]).astype(np.int64)
    text_table = f32(inputs["text_table"])
    audio_table = f32(inputs["audio_table"])
    sa_in_w = f32(inputs["sa_in_w"])
    sa_out_w = f32(inputs["sa_out_w"])
    ca_in_w = f32(inputs["ca_in_w"])
    ca_out_w = f32(inputs["ca_out_w"])
    ffn_w1 = f32(inputs["ffn_w1"])
    ffn_w2 = f32(inputs["ffn_w2"])

    pe_t = _pe_table(Tt, D)
    pe_a = _pe_table(Ta, D)

    # per-TP-rank weight shards (shared by the 4 cores with the same rank)
    shard = []
    for r in range(2):
        sl = slice(512 * r, 512 * (r + 1))
        fl = slice(2048 * r, 2048 * (r + 1))
        wq, wk, wv = (sa_in_w[:NLAYERS, i * D:(i + 1) * D, :][:, sl]
                      for i in range(3))
        cq, ck, cv = (ca_in_w[:NLAYERS, i * D:(i + 1) * D, :][:, sl]
                      for i in range(3))
        hw = np.zeros((HEADO * P, D), np.float32)
        hw[0:513] = audio_table[513 * r:513 * (r + 1)]
        # interleave q/k 128-row groups so the device's first o-block of the
        # qk projection yields heads 0/1's q AND k (attention starts early)
        qk_il = np.concatenate(
            [np.concatenate([wq[:, i * P:(i + 1) * P, :],
                             wk[:, i * P:(i + 1) * P, :]], axis=1)
             for i in range(4)], axis=1)
        shard.append({
            "wqk_sa": _bf(qk_il.transpose(0, 2, 1)),
            "wv_sa": _bf(wv.transpose(0, 2, 1)),
            "wo_sa": _bf(sa_out_w[:NLAYERS, :, sl].transpose(0, 2, 1)),
            "wq_ca": _bf(cq.transpose(0, 2, 1)),
            "wk_ca": _bf(ck.transpose(0, 2, 1)),
            "wv_ca": _bf(cv.transpose(0, 2, 1)),
            "wo_ca": _bf(ca_out_w[:NLAYERS, :, sl].transpose(0, 2, 1)),
            "w1T": _bf(ffn_w1[:NLAYERS, fl, :].transpose(0, 2, 1)),
            "w2T": _bf(ffn_w2[:NLAYERS, :, fl].transpose(0, 2, 1)),
            "headT": _bf(hw.T),
        })

    in_maps = []
    for c in range(8):
        p, r = c // 2, c % 2
        tlb, alb = int(tl[p]), int(al[p])
        il = tlb + alb

        emb = np.zeros((L, D), np.float32)
        emb[:tlb] = text_table[text[p, :tlb]] + pe_t[:tlb]
        emb[tlb:il] = audio_table[audio[p, :alb]] + pe_a[:alb]
        x0 = np.ascontiguousarray(emb.T)

        kk = np.arange(L)
        allowed = ((kk[None, :] < tlb)
                   | ((kk[None, :] <= kk[:, None]) & (kk[:, None] < il)))
        # maskT[k_part, tk*L + q] = allowed(q, tk*128 + k_part)
        mt = allowed.T.reshape(NT, P, L).transpose(1, 0, 2).reshape(P, NT * L)

        m = {"x0": x0, "mem0": _bf(x0), "maskT": _bf(mt)}
        m.update(shard[r])
        in_maps.append(m)

    key = (NLAYERS, os.environ.get("KERNEL_DEBUG", "0"))
    if _NC_CACHE.get("key") != key:
        _NC_CACHE["nc"] = _build_nc()
        _NC_CACHE["key"] = key
    nc = _NC_CACHE["nc"]
    trace = bool(int(os.environ.get("KERNEL_TRACE", "0")))
    r = run_bass_kernel_spmd(nc, in_maps, core_ids=list(range(8)), trace=trace)
    LAST_RESULT["r"] = r
    res = r.results

    out = np.empty((B, L, VA), np.float32)
    for p in range(B):
        ev = np.asarray(res[2 * p]["logits"], dtype=np.float32)
        od = np.asarray(res[2 * p + 1]["logits"], dtype=np.float32)
        out[p] = np.concatenate([ev[0:513], od[0:513]], axis=0).T
    return out
